# revision 37
# baseline (speedup 1.0000x reference)
"""Trainium2 Bass/Tile kernel for a ViT block with routed sparse attention.

Strategy (hardcoded for B=32, S=197, D=768, H=12, P=196, k=32):
  - Data-parallel over batch: 8 cores x 4 batch elements, weights replicated.
  - The routed sparse attention is computed densely with a host-precomputed
    count matrix C[s, q] (= multiplicity of key-token s among query q's routed
    neighbors; CLS row/col handled specially).  softmax over gathered
    neighbors (with duplicates) == C-weighted dense softmax.
  - bf16 matmul inputs, fp32 PSUM accumulation, fp32 residual stream.
  - LayerNorm gains/biases folded into the following matmul weights on host.
"""

import threading
from contextlib import ExitStack

import ml_dtypes
import numpy as np

import concourse.bass as bass
import concourse.bacc as bacc
import concourse.mybir as mybir
import concourse.tile as tile
from concourse.bass_utils import run_bass_kernel_spmd

AF = mybir.ActivationFunctionType
ALU = mybir.AluOpType
dt = mybir.dt
BF16 = ml_dtypes.bfloat16

B, S, D, H = 32, 197, 768, 12
PN, KN, HD, DFF = 196, 32, 64, 3072
NCORES = 8
BL = B // NCORES  # 4 batch elements per core
T = BL * S  # 788 tokens per core
KD = D // 128  # 6 contraction tiles over D
KF = DFF // 128  # 24 contraction tiles over DFF
SCALE = HD**-0.5  # 0.125

# token tiles aligned with the residual stream layout
TTS = [(i * 128, min(128, T - i * 128)) for i in range((T + 127) // 128)]
# free-dim chunks over T for moving operands (PSUM holds <=512 fp32)
TCH = [(0, 512), (512, T - 512)]
# free-dim chunks over D
DCH = [(0, 512), (512, 256)]
# per-batch s-tiles for attention (S = 128 + 69)
SST = [(0, 128), (128, S - 128)]
# last batch whose attention output feeds proj of token tile ti
TT_BATCH = [(o + sz - 1) // S for (o, sz) in TTS]


def _emit(nc):
    f32, bf16 = dt.float32, dt.bfloat16
    x_d = nc.dram_tensor("x", [T, D], f32, kind="ExternalInput")
    ct_d = nc.dram_tensor("ct", [S, S], bf16, kind="ExternalInput")
    wqk_d = nc.dram_tensor("wqk", [D, 2 * D], bf16, kind="ExternalInput")
    wv_d = nc.dram_tensor("wv", [D, D], bf16, kind="ExternalInput")
    wp_d = nc.dram_tensor("wp", [D, D], bf16, kind="ExternalInput")
    w1_d = nc.dram_tensor("w1", [D, DFF], bf16, kind="ExternalInput")
    w2_d = nc.dram_tensor("w2", [DFF, D], bf16, kind="ExternalInput")
    bqk_d = nc.dram_tensor("bqk", [128, 2 * KD], f32, kind="ExternalInput")
    b1_d = nc.dram_tensor("b1", [128, KF], f32, kind="ExternalInput")
    bp_d = nc.dram_tensor("bp", [D], bf16, kind="ExternalInput")
    b2_d = nc.dram_tensor("b2", [D], bf16, kind="ExternalInput")
    eye_d = nc.dram_tensor("eye", [128, 128], bf16, kind="ExternalInput")
    out_d = nc.dram_tensor("out", [T, D], f32, kind="ExternalOutput")

    with tile.TileContext(nc, pool_alloc_mode="queue") as tc, ExitStack() as ctx:
        const = ctx.enter_context(tc.tile_pool(name="const", bufs=1))
        px = ctx.enter_context(tc.tile_pool(name="px", bufs=len(TTS)))
        pzt = ctx.enter_context(tc.tile_pool(name="pzt", bufs=KD))
        pv = ctx.enter_context(tc.tile_pool(name="pv", bufs=BL * 2))
        ph = ctx.enter_context(tc.tile_pool(name="ph", bufs=KF))
        psm = ctx.enter_context(tc.tile_pool(name="psm", bufs=3))
        pew = ctx.enter_context(tc.tile_pool(name="pew", bufs=4))
        # manually released pools, allocation order = reverse release order
        pwp = tc.alloc_tile_pool(name="pwp", bufs=KD)
        pat = tc.alloc_tile_pool(name="pat", bufs=KD)
        pqk = tc.alloc_tile_pool(name="pqk", bufs=2 * KD)
        pwv = tc.alloc_tile_pool(name="pwv", bufs=KD)
        pwqk = tc.alloc_tile_pool(name="pwqk", bufs=KD)

        # ---- DMAs in latency-priority order ----
        eye_sb = const.tile([128, 128], bf16, tag="eye")
        nc.sync.dma_start(out=eye_sb, in_=eye_d[:, :])
        xts = []
        for ti, (o, sz) in enumerate(TTS):
            x_t = px.tile([128, D], f32, name=f"x{ti}", tag="x", bufs=len(TTS))
            nc.sync.dma_start(out=x_t[:sz, :], in_=x_d[o : o + sz, :])
            xts.append(x_t)
        wv_sb = []
        for k in range(KD):
            w_t = pwv.tile([128, D], bf16, name=f"wv{k}", tag="wv", bufs=KD)
            nc.sync.dma_start(out=w_t, in_=wv_d[k * 128 : (k + 1) * 128, :])
            wv_sb.append(w_t)
        wqk_sb = []
        for k in range(KD):
            w_t = pwqk.tile([128, 2 * D], bf16, name=f"wqk{k}", tag="wqk", bufs=KD)
            nc.sync.dma_start(out=w_t, in_=wqk_d[k * 128 : (k + 1) * 128, :])
            wqk_sb.append(w_t)
        wp_sb = []
        for k in range(KD):
            w_t = pwp.tile([128, D], bf16, name=f"wp{k}", tag="wp", bufs=KD)
            nc.sync.dma_start(out=w_t, in_=wp_d[k * 128 : (k + 1) * 128, :])
            wp_sb.append(w_t)
        # count matrix, both s-tiles packed as [128, 2, S] (rows 69.. of slot 1
        # are never read)
        ct_sb = const.tile([128, 2, S], bf16, tag="ct")
        nc.sync.dma_start(out=ct_sb[:128, 0, :], in_=ct_d[0:128, :])
        nc.sync.dma_start(out=ct_sb[: S - 128, 1, :], in_=ct_d[128:S, :])
        bqk_sb = const.tile([128, 2 * KD], f32, tag="bqk")
        nc.sync.dma_start(out=bqk_sb, in_=bqk_d[:, :])
        b1_sb = const.tile([128, KF], f32, tag="b1")
        nc.sync.dma_start(out=b1_sb, in_=b1_d[:, :])
        bp_sb = const.tile([1, D], bf16, tag="bp")
        nc.sync.dma_start(out=bp_sb, in_=bp_d[:].rearrange("(a d) -> a d", a=1))
        b2_sb = const.tile([1, D], bf16, tag="b2")
        nc.sync.dma_start(out=b2_sb, in_=b2_d[:].rearrange("(a d) -> a d", a=1))
        ones_sb = const.tile([128, 128], bf16, tag="ones")
        nc.vector.memset(ones_sb, 1.0)
        eps_sb = const.tile([128, 1], f32, tag="eps")
        nc.vector.memset(eps_sb, 1e-5)
        # pre-touch ACT-consumed DMA'd tiles (TRN2 allows 1 sync wait per
        # instruction; keeping table-load-carrying ACT ops at <=1 producer
        # engine avoids walrus sync-slot overflow) and warm the ln/exp table
        act_touch = const.tile([128, 1], f32, tag="act_touch")
        nc.scalar.copy(out=act_touch, in_=b1_sb[:, 0:1])
        act_warm = const.tile([128, 1], f32, tag="act_warm")
        nc.scalar.activation(out=act_warm, in_=eps_sb, func=AF.Exp)

        def layer_norm_stats(ti):
            """standardize x tile (fp32, [t,d]) -> zn bf16 [t,d]"""
            o, sz = TTS[ti]
            stats = psm.tile([128, 3, 6], f32, tag="stats", bufs=3)
            for g in range(3):
                nc.vector.bn_stats(
                    out=stats[:sz, g, :], in_=xts[ti][:sz, g * 256 : (g + 1) * 256]
                )
            mv = psm.tile([128, 2], f32, tag="mv", bufs=3)
            nc.vector.bn_aggr(out=mv[:sz], in_=stats[:sz])
            # rstd = exp(-0.5 * ln(var + eps)) -- Ln/Exp share one ACT
            # table set with the attention exp, avoiding table thrash
            nc.scalar.activation(
                out=mv[:sz, 1:2], in_=mv[:sz, 1:2], func=AF.Ln,
                bias=eps_sb[:sz], scale=1.0,
            )
            nc.scalar.activation(
                out=mv[:sz, 1:2], in_=mv[:sz, 1:2], func=AF.Exp, scale=-0.5,
            )
            zn = psm.tile([128, D], bf16, tag="zn", bufs=8)
            nc.vector.tensor_scalar(
                out=zn[:sz, :], in0=xts[ti][:sz, :],
                scalar1=mv[:sz, 0:1], scalar2=mv[:sz, 1:2],
                op0=ALU.subtract, op1=ALU.mult,
            )
            return zn

        def zt_transposes(ti, zn, zt_tiles, tr_pool, tr_bufs):
            o, sz = TTS[ti]
            for k in range(KD):
                pt = tr_pool.tile([128, 128], bf16, tag="mm", bufs=tr_bufs)
                nc.tensor.transpose(
                    out=pt[:, :sz],
                    in_=zn[:sz, k * 128 : (k + 1) * 128],
                    identity=eye_sb[:sz, :sz],
                )
                if k % 2 == 0:
                    nc.vector.tensor_copy(out=zt_tiles[k][:, o : o + sz], in_=pt[:, :sz])
                else:
                    nc.scalar.copy(out=zt_tiles[k][:, o : o + sz], in_=pt[:, :sz])

        def layer_norm_to_zt(ti, zt_tiles, tr_pool, tr_bufs):
            zn = layer_norm_stats(ti)
            zt_transposes(ti, zn, zt_tiles, tr_pool, tr_bufs)

        # ---- P1: LN1 -> z1T, V interleaved per batch ----
        ps_a = tc.alloc_tile_pool(name="ps_a", bufs=3, space="PSUM")
        ps_tr1 = tc.alloc_tile_pool(name="ps_tr1", bufs=2, space="PSUM")
        z1t = [pzt.tile([128, T], bf16, name=f"z1t{k}", tag="zt", bufs=KD) for k in range(KD)]
        vaug = [
            [pv.tile([ss, 65 * H], bf16, name=f"v{b}_{st}", tag="v", bufs=BL * 2)
             for st, (so, ss) in enumerate(SST)]
            for b in range(BL)
        ]

        def v_batch(b):
            """V for batch b (normal layout, heads strided by 65 with a ones
            column for the softmax denominator)"""
            for st, (so, ss) in enumerate(SST):
                va = vaug[b][st].rearrange("p (h c) -> p h c", c=65)
                nc.vector.memset(va[:, :, 64:65], 1.0)
                for off, cs in DCH:
                    mm = ps_a.tile([128, 512], f32, tag="mm", bufs=3)
                    for k in range(KD):
                        nc.tensor.matmul(
                            mm[:ss, :cs],
                            z1t[k][:, b * S + so : b * S + so + ss],
                            wv_sb[k][:, off : off + cs],
                            start=(k == 0),
                            stop=(k == KD - 1),
                        )
                    nc.scalar.copy(
                        out=va[:ss, off // 64 : (off + cs) // 64, 0:64],
                        in_=mm[:ss, :cs].rearrange("p (h c) -> p h c", c=64),
                    )

        # LN1 tile i unlocks V of the batches its rows complete
        layer_norm_to_zt(0, z1t, ps_tr1, 2)
        layer_norm_to_zt(1, z1t, ps_tr1, 2)
        v_batch(0)
        layer_norm_to_zt(2, z1t, ps_tr1, 2)
        layer_norm_to_zt(3, z1t, ps_tr1, 2)
        v_batch(1)
        layer_norm_to_zt(4, z1t, ps_tr1, 2)
        v_batch(2)
        layer_norm_to_zt(5, z1t, ps_tr1, 2)
        layer_norm_to_zt(6, z1t, ps_tr1, 2)
        v_batch(3)
        ps_tr1.release()

        # ---- QKT with attention interleaved per head-pair: pair j's softmax
        #      (ACT/DVE) overlaps pair j+1's dense matmuls (PE) ----
        qt = [pqk.tile([128, T], bf16, name=f"qt{k}", tag="qk", bufs=2 * KD) for k in range(KD)]
        kt = [pqk.tile([128, T], bf16, name=f"kt{k}", tag="qk", bufs=2 * KD) for k in range(KD)]
        att = [pat.tile([128, T], bf16, name=f"att{k}", tag="at", bufs=KD) for k in range(KD)]
        ps_att = tc.alloc_tile_pool(name="ps_att", bufs=3, space="PSUM")

        def attention_pair(b, j):
            base = b * S
            avp = ps_att.tile([65, 2, S], f32, tag="avp", bufs=3)
            for hh in range(2):
                h = 2 * j + hh
                r0 = 64 * hh
                scp = ps_att.tile([128, 2, S], f32, tag="scp", bufs=2)
                for st, (so, ss) in enumerate(SST):
                    # scores^T[s, q] = K_h^T.T @ Q_h^T (contract over hd)
                    nc.tensor.matmul(
                        scp[:ss, st, :],
                        kt[j][r0 : r0 + 64, base + so : base + so + ss],
                        qt[j][r0 : r0 + 64, base : base + S],
                        start=True,
                        stop=True,
                    )
                # exp over both s-tiles in one op (rows 69.. of slot 1 are
                # garbage and never read downstream)
                ew = pew.tile([128, 2, S], bf16, tag="ew", bufs=8)
                nc.scalar.activation(out=ew, in_=scp[:, :, :], func=AF.Exp, scale=SCALE)
                # count-multiply: alternate DVE / GPSIMD to split the load
                wt = pew.tile([128, 2, S], bf16, tag="wt", bufs=8)
                if (j + hh) % 2 == 0:
                    nc.vector.tensor_tensor(out=wt, in0=ew, in1=ct_sb, op=ALU.mult)
                else:
                    nc.gpsimd.tensor_mul(out=wt, in0=ew, in1=ct_sb)
                # out_un^T[d|1, q] = [V_h | 1]^T @ W^T ; row 64 = denominator
                for st, (so, ss) in enumerate(SST):
                    nc.tensor.matmul(
                        avp[:, hh, :],
                        vaug[b][st][:ss, h * 65 : (h + 1) * 65],
                        wt[:ss, st, :],
                        start=(st == 0),
                        stop=(st == 1),
                    )
            # 1/D for both heads, broadcast over 64 partitions via K=1 matmul
            recd = pew.tile([65, 2, S], bf16, tag="recd", bufs=4)
            nc.vector.reciprocal(out=recd[64:65, :, :], in_=avp[64:65, :, :])
            bcp = ps_att.tile([64, 2, S], f32, tag="avp", bufs=3)
            nc.tensor.matmul(
                bcp[:, :, :],
                ones_sb[64:65, 0:64],
                recd[64:65, :, :].rearrange("a b c -> a (b c)"),
                start=True,
                stop=True,
            )
            bcs = pew.tile([64, 2, S], f32, tag="bcs", bufs=4)
            nc.scalar.copy(out=bcs, in_=bcp[:, :, :])
            nc.vector.tensor_tensor(
                out=att[j][0:64, base : base + S],
                in0=avp[0:64, 0, :], in1=bcs[:, 0, :], op=ALU.mult,
            )
            atmp = pew.tile([64, S], bf16, tag="atmp", bufs=4)
            nc.vector.tensor_tensor(
                out=atmp, in0=avp[0:64, 1, :], in1=bcs[:, 1, :], op=ALU.mult
            )
            nc.sync.dma_start(out=att[j][64:128, base : base + S], in_=atmp)

        for j in range(KD):
            for ft in (j, j + KD):
                dst = qt[ft] if ft < KD else kt[ft - KD]
                for off, cs in TCH:
                    mm = ps_a.tile([128, 512], f32, tag="mm", bufs=3)
                    for k in range(KD):
                        nc.tensor.matmul(
                            mm[:, :cs],
                            wqk_sb[k][:, ft * 128 : (ft + 1) * 128],
                            z1t[k][:, off : off + cs],
                            start=(k == 0),
                            stop=(k == KD - 1),
                        )
                    if ft % 2 == 0:
                        nc.vector.tensor_scalar_add(
                            out=dst[:, off : off + cs],
                            in0=mm[:, :cs],
                            scalar1=bqk_sb[:, ft : ft + 1],
                        )
                    else:
                        nc.scalar.activation(
                            out=dst[:, off : off + cs],
                            in_=mm[:, :cs],
                            func=AF.Identity,
                            bias=bqk_sb[:, ft : ft + 1],
                            scale=1.0,
                        )
            for b in range(BL):
                attention_pair(b, j)

        ps_att.release()
        ps_a.release()
        pwqk.release()
        pwv.release()

        # w1 resident (36KB/partition are free by now); prefetched during proj
        pw1s = tc.alloc_tile_pool(name="pw1s", bufs=KD * KD)
        w1_sb = []
        for k in range(KD):
            w_t = pw1s.tile([128, DFF], bf16, name=f"w1r{k}", tag="w1", bufs=KD)
            nc.sync.dma_start(out=w_t, in_=w1_d[k * 128 : (k + 1) * 128, :])
            w1_sb.append(w_t)

        # ---- proj + residual + LN2 per token tile ----
        ps_b = tc.alloc_tile_pool(name="ps_b", bufs=6, space="PSUM")
        z2t = [pzt.tile([128, T], bf16, name=f"z2t{k}", tag="zt", bufs=KD) for k in range(KD)]

        def proj_tile(ti):
            o, sz = TTS[ti]
            for off, cs in DCH:
                mm = ps_b.tile([128, 512], f32, tag="mm", bufs=6)
                for k in range(KD):
                    nc.tensor.matmul(
                        mm[:sz, :cs],
                        att[k][:, o : o + sz],
                        wp_sb[k][:, off : off + cs],
                        start=(k == 0),
                        stop=False,
                    )
                nc.tensor.matmul(
                    mm[:sz, :cs],
                    ones_sb[0:1, :sz],
                    bp_sb[0:1, off : off + cs],
                    start=False,
                    stop=True,
                )
                nc.vector.tensor_tensor(
                    out=xts[ti][:sz, off : off + cs],
                    in0=xts[ti][:sz, off : off + cs],
                    in1=mm[:sz, :cs],
                    op=ALU.add,
                )
            # LN2 stats right away; transposes are batched by the caller
            return layer_norm_stats(ti)

        PROJ_SPLIT = 4
        zns = {}
        for ti in range(PROJ_SPLIT):
            zns[ti] = proj_tile(ti)
        for ti in range(PROJ_SPLIT):
            zt_transposes(ti, zns[ti], z2t, ps_b, 6)

        ht = [ph.tile([128, T], bf16, name=f"ht{k}", tag="ht", bufs=KF) for k in range(KF)]
        FTB = 4  # ft per w1 column block (512 cols)

        def mlp1_chunk(ci, mm_pool, mm_bufs):
            off, cs = TCH[ci]
            for ft in range(KF):
                mm = mm_pool.tile([128, 512], f32, tag="mm", bufs=mm_bufs)
                for k in range(KD):
                    nc.tensor.matmul(
                        mm[:, :cs],
                        w1_sb[k][:, ft * 128 : (ft + 1) * 128],
                        z2t[k][:, off : off + cs],
                        start=(k == 0),
                        stop=(k == KD - 1),
                    )
                nc.scalar.activation(
                    out=ht[ft][:, off : off + cs],
                    in_=mm[:, :cs],
                    func=AF.Gelu,
                    bias=b1_sb[:, ft : ft + 1],
                    scale=1.0,
                )

        mlp1_chunk(0, ps_b, 6)
        for ti in range(PROJ_SPLIT, len(TTS)):
            zns[ti] = proj_tile(ti)
        for ti in range(PROJ_SPLIT, len(TTS)):
            zt_transposes(ti, zns[ti], z2t, ps_b, 6)
        ps_b.release()
        ps_c = tc.alloc_tile_pool(name="ps_c", bufs=8, space="PSUM")
        mlp1_chunk(1, ps_c, 8)
        pw1s.release()
        pqk.release()
        pat.release()
        pwp.release()
        # ---- MLP2: x <- x + h @ w2 + b2 ; store ----
        pw2 = tc.alloc_tile_pool(name="pw2", bufs=KF)
        w2_sb = []
        for k in range(KF):
            w_t = pw2.tile([128, D], bf16, name=f"w2{k}", tag="w2", bufs=KF)
            nc.sync.dma_start(out=w_t, in_=w2_d[k * 128 : (k + 1) * 128, :])
            w2_sb.append(w_t)
        for ti, (o, sz) in enumerate(TTS):
            for off, cs in DCH:
                mm = ps_c.tile([128, 512], f32, tag="mm", bufs=8)
                for k in range(KF):
                    nc.tensor.matmul(
                        mm[:sz, :cs],
                        ht[k][:, o : o + sz],
                        w2_sb[k][:, off : off + cs],
                        start=(k == 0),
                        stop=False,
                    )
                nc.tensor.matmul(
                    mm[:sz, :cs],
                    ones_sb[0:1, :sz],
                    b2_sb[0:1, off : off + cs],
                    start=False,
                    stop=True,
                )
                nc.vector.tensor_tensor(
                    out=xts[ti][:sz, off : off + cs],
                    in0=xts[ti][:sz, off : off + cs],
                    in1=mm[:sz, :cs],
                    op=ALU.add,
                )
                nc.sync.dma_start(
                    out=out_d[o : o + sz, off : off + cs],
                    in_=xts[ti][:sz, off : off + cs],
                )
        ps_c.release()
        pw2.release()

    return nc


_nc_lock = threading.Lock()
_nc_cache = {}


def _constrain_act_tables():
    """Make Bacc's table-load inserter place Ln/Exp/Copy in the shared
    natural_log_exp_and_others set (and Gelu in gelu_and_others) instead of
    ping-ponging between per-function sets. Indices into act_info.json are
    preserved; we only empty the other sets so they can't be chosen."""
    import concourse.hw_specs as hw_specs

    orig = hw_specs.get_activation_tables
    keep = {"natural_log_exp_and_others", "gelu_and_others"}

    def patched(arch):
        tabs = orig(arch)
        return {k: (set(v) if k in keep else set()) for k, v in tabs.items()}

    bacc.get_activation_tables = patched


def _get_nc():
    with _nc_lock:
        if "nc" not in _nc_cache:
            _constrain_act_tables()
            nc = bacc.Bacc("TRN2", target_bir_lowering=False)
            with nc.allow_low_precision(reason="bf16 softmax reciprocal broadcast"):
                _emit(nc)
            nc.finalize()
            _nc_cache["nc"] = nc
        return _nc_cache["nc"]


def _prep_inputs(inputs):
    x = np.asarray(inputs["x"], np.float32)
    routes = np.asarray(inputs["routes"], np.int64)
    qkv_w = np.asarray(inputs["qkv_w"], np.float32)
    qkv_b = np.asarray(inputs["qkv_b"], np.float32)
    proj_w = np.asarray(inputs["proj_w"], np.float32)
    proj_b = np.asarray(inputs["proj_b"], np.float32)
    n1_g = np.asarray(inputs["n1_g"], np.float32)
    n1_b = np.asarray(inputs["n1_b"], np.float32)
    n2_g = np.asarray(inputs["n2_g"], np.float32)
    n2_b = np.asarray(inputs["n2_b"], np.float32)
    mlp_w1 = np.asarray(inputs["mlp_w1"], np.float32)
    mlp_b1 = np.asarray(inputs["mlp_b1"], np.float32)
    mlp_w2 = np.asarray(inputs["mlp_w2"], np.float32)
    mlp_b2 = np.asarray(inputs["mlp_b2"], np.float32)

    # count matrix: CT[s, q] = multiplicity of key-token s for query-token q
    ct = np.zeros((S, S), np.float32)
    np.add.at(
        ct,
        (routes.reshape(-1) + 1, np.repeat(np.arange(PN), KN) + 1),
        1.0,
    )
    ct[:, 0] = 1.0  # CLS query attends densely to every token

    # fold LN1 gain into qkv weights, LN1 shift into qkv bias
    wqk = (qkv_w[:, : 2 * D] * n1_g[:, None]).astype(BF16)
    bqk = (n1_b @ qkv_w[:, : 2 * D] + qkv_b[: 2 * D]).astype(np.float32)
    wv = (qkv_w[:, 2 * D :] * n1_g[:, None]).astype(BF16)
    bv = n1_b @ qkv_w[:, 2 * D :] + qkv_b[2 * D :]
    # softmax rows sum to 1 -> V bias lands additively on attn out; fold into
    # the proj bias
    bp = (proj_b + bv @ proj_w).astype(BF16)
    w1 = (mlp_w1 * n2_g[:, None]).astype(BF16)
    b1 = (n2_b @ mlp_w1 + mlp_b1).astype(np.float32)

    shared = {
        "ct": ct.astype(BF16),
        "wqk": wqk,
        "wv": wv,
        "wp": proj_w.astype(BF16),
        "w1": w1,
        "w2": mlp_w2.astype(BF16),
        # per-partition bias layout [128, n_tiles], contiguous for the DMA
        "bqk": np.ascontiguousarray(bqk.reshape(2 * KD, 128).T),
        "b1": np.ascontiguousarray(b1.reshape(KF, 128).T),
        "bp": bp,
        "b2": mlp_b2.astype(BF16),
        "eye": np.eye(128, dtype=BF16),
    }
    in_maps = []
    for c in range(NCORES):
        m = dict(shared)
        m["x"] = np.ascontiguousarray(x[c * BL : (c + 1) * BL].reshape(T, D))
        in_maps.append(m)
    return in_maps


def run(inputs, trace=False):
    nc = _get_nc()
    in_maps = _prep_inputs(inputs)
    res = run_bass_kernel_spmd(
        nc, in_maps, core_ids=list(range(NCORES)), trace=trace
    )
    out = np.concatenate(
        [r["out"].reshape(BL, S, D) for r in res.results], axis=0
    ).astype(np.float32)
    return out, res


def kernel(**inputs):
    out, _ = run(inputs, trace=False)
    return out
